# revision 1
# baseline (speedup 1.0000x reference)
"""Trainium2 Bass kernel for CrossAttention (b=4, p=8, n=512, dim=512, 8 heads x 64).

Sharding: the 32 independent (b, p) slices are split 4-per-core across 8
NeuronCores (pure data parallel, no collectives). Weights are replicated.

Host-side prep (inside kernel()): inputs are transposed per-slice to
[dim, n] and cast to bf16, so the device never transposes activations;
weights are cast to bf16 on the host too.

Per-slice device dataflow (all SBUF tiles are [partition, free]):
  - qT = Wq-blocks^T @ xqT, kT likewise; v = xkvT-blocks^T @ Wv  (PE)
  - per head: ST[j, i] = kT_h-block^T @ qT_h -> exp (ACT, scale=1/8) -> PT bf16
    (no max subtraction: scores are ~N(0,1), exp cannot overflow)
  - PV for a head pair is issued column-group-paired so PE overlaps:
    even head outT -> PSUM[0:64] while odd head's l-matmul (ones[128,33])
    lands in PSUM[64:97] of a second bank, and vice versa. l rows at the
    two quadrant bases let a DVE stream_shuffle broadcast l across all 64
    partitions of the head's parity range, all partition-aligned.
  - normalize: outT_h *= 1/l_h (DVE reciprocal + stream_shuffle + mul)
  - final: fin[i, f] = outT-blocks^T @ Wo (+ ones1 x bo) -> fp32 -> DRAM.
"""

from contextlib import ExitStack

import ml_dtypes
import numpy as np

import concourse.bass as bass
import concourse.tile as tile
from concourse import bacc, mybir
from concourse.bass_utils import run_bass_kernel_spmd

F32 = mybir.dt.float32
BF16 = mybir.dt.bfloat16

HEADS = 8
DH = 64
N = 512
DIM = 512
SCALE = DH**-0.5
S = 4  # (b, p) slices per core
N_CORES = 8

SHUF0 = [0] * 32  # stream_shuffle mask: broadcast quadrant partition 0


def _build_body(ctx: ExitStack, tc: tile.TileContext, qT, kvT, wq, wk, wv, wo, bo, out):
    nc = tc.nc

    const = ctx.enter_context(tc.tile_pool(name="const", bufs=1))
    xT = ctx.enter_context(tc.tile_pool(name="xT", bufs=3))
    proj = ctx.enter_context(tc.tile_pool(name="proj", bufs=2))
    ptp = ctx.enter_context(tc.tile_pool(name="ptp", bufs=4))
    outTp = ctx.enter_context(tc.tile_pool(name="outTp", bufs=2))
    rbp = ctx.enter_context(tc.tile_pool(name="rbp", bufs=4))
    finp = ctx.enter_context(tc.tile_pool(name="finp", bufs=2))
    mm_ps = ctx.enter_context(tc.tile_pool(name="mm_ps", bufs=2, space="PSUM"))
    st_ps = ctx.enter_context(tc.tile_pool(name="st_ps", bufs=3, space="PSUM"))
    pv_ps = ctx.enter_context(tc.tile_pool(name="pv_ps", bufs=2, space="PSUM"))
    l_ps = ctx.enter_context(tc.tile_pool(name="l_ps", bufs=1, space="PSUM"))

    # --- weights (already bf16 in DRAM): [512, 512] -> [128, 4*512] ---
    w_sb = {}
    for name, dram in (("wq", wq), ("wk", wk), ("wv", wv), ("wo", wo)):
        w16 = const.tile([128, 4 * 512], BF16, name=f"{name}16")
        nc.sync.dma_start(w16[:], dram.rearrange("(t p) e -> p t e", p=128))
        w_sb[name] = w16
    bo16 = const.tile([1, 512], BF16, name="bo16")
    nc.sync.dma_start(bo16[:], bo.rearrange("(o f) -> o f", o=1))
    ones64 = const.tile([128, 64], BF16, name="ones64")
    nc.gpsimd.memset(ones64[:], 1.0)
    ones1 = const.tile([1, 128], BF16, name="ones1")
    nc.gpsimd.memset(ones1[:], 1.0)
    wq16, wk16, wv16, wo16 = (w_sb[k] for k in ("wq", "wk", "wv", "wo"))

    for s in range(S):
        # --- load pre-transposed bf16 inputs ---
        xqT = xT.tile([128, 4 * 512], BF16, name="xqT")
        nc.sync.dma_start(xqT[:], qT[s].rearrange("(t p) n -> p t n", p=128))
        xkvT = xT.tile([128, 4 * 512], BF16, name="xkvT")
        nc.sync.dma_start(xkvT[:], kvT[s].rearrange("(t p) n -> p t n", p=128))

        # --- projections ---
        qT16 = proj.tile([128, 4 * 512], BF16, name="qT16")
        kT16 = proj.tile([128, 4 * 512], BF16, name="kT16")
        v16 = proj.tile([128, 4 * 512], BF16, name="v16")
        for w16, xt, dst in ((wq16, xqT, qT16), (wk16, xkvT, kT16)):
            for t in range(4):  # output row-block (e)
                ps = mm_ps.tile([128, 512], F32, name="mm_ps")
                for d in range(4):  # contraction block
                    nc.tensor.matmul(
                        ps[:],
                        w16[:, d * 512 + t * 128 : d * 512 + (t + 1) * 128],
                        xt[:, d * 512 : (d + 1) * 512],
                        start=(d == 0),
                        stop=(d == 3),
                    )
                nc.vector.tensor_copy(dst[:, t * 512 : (t + 1) * 512], ps[:])
        for jb in range(4):  # v, normal layout: rows j, free e
            ps = mm_ps.tile([128, 512], F32, name="mm_ps")
            for d in range(4):
                nc.tensor.matmul(
                    ps[:],
                    xkvT[:, d * 512 + jb * 128 : d * 512 + (jb + 1) * 128],
                    wv16[:, d * 512 : (d + 1) * 512],
                    start=(d == 0),
                    stop=(d == 3),
                )
            nc.vector.tensor_copy(v16[:, jb * 512 : (jb + 1) * 512], ps[:])

        # --- attention, head pairs ---
        outT16 = outTp.tile([128, 4 * 512], BF16, name="outT16")
        for tp in range(4):
            h0, h1 = 2 * tp, 2 * tp + 1
            pts = []
            for h, half in ((h0, 0), (h1, 64)):
                kT_h = kT16[half : half + 64, tp * 512 : (tp + 1) * 512]
                qT_h = qT16[half : half + 64, tp * 512 : (tp + 1) * 512]
                pt16 = ptp.tile([128, 4 * 512], BF16, name="pt16")
                for jb in range(4):
                    stt = st_ps.tile([128, 512], F32, name="st_ps")
                    nc.tensor.matmul(
                        stt[:],
                        kT_h[:, jb * 128 : (jb + 1) * 128],
                        qT_h,
                        start=True,
                        stop=True,
                    )
                    nc.scalar.activation(
                        pt16[:, jb * 512 : (jb + 1) * 512],
                        stt[:],
                        mybir.ActivationFunctionType.Exp,
                        scale=SCALE,
                    )
                pts.append(pt16)
            pt_e, pt_o = pts

            # PV: column-group-paired issues so PE overlaps outT with l.
            pv = pv_ps.tile([128, 512], F32, name="pv_ps")
            lps = l_ps.tile([128, 512], F32, name="l_ps")
            for jb in range(4):
                pe_s = pt_e[:, jb * 512 : (jb + 1) * 512]
                po_s = pt_o[:, jb * 512 : (jb + 1) * 512]
                st, sp = (jb == 0), (jb == 3)
                # issue A: even outT (cols 0-63) || odd l-bcast (cols 64-127)
                nc.tensor.matmul(
                    pv[0:64, :],
                    v16[:, jb * 512 + h0 * 64 : jb * 512 + (h0 + 1) * 64],
                    pe_s, start=st, stop=sp, skip_group_check=True,
                )
                nc.tensor.matmul(
                    lps[64:128, :], ones64[:], po_s, start=st, stop=sp,
                    skip_group_check=True,
                )
                # issue B: odd outT (cols 64-127) || even l-bcast (cols 0-63)
                nc.tensor.matmul(
                    pv[64:128, :],
                    v16[:, jb * 512 + h1 * 64 : jb * 512 + (h1 + 1) * 64],
                    po_s, start=st, stop=sp, skip_group_check=True,
                )
                nc.tensor.matmul(
                    lps[0:64, :], ones64[:], pe_s, start=st, stop=sp,
                    skip_group_check=True,
                )
            for h, half in ((h0, 0), (h1, 64)):
                rb1 = rbp.tile([128, 512], F32, name="rb1")
                nc.vector.reciprocal(
                    rb1[half : half + 64, :], lps[half : half + 64, :]
                )
                nc.vector.tensor_mul(
                    outT16[half : half + 64, tp * 512 : (tp + 1) * 512],
                    pv[half : half + 64, :],
                    rb1[half : half + 64, :],
                )

        # --- final projection + bias ---
        fin = finp.tile([128, 4 * 512], F32, name="fin")
        for ib in range(4):
            ps = mm_ps.tile([128, 512], F32, name="mm_ps")
            for t in range(4):
                nc.tensor.matmul(
                    ps[:],
                    outT16[:, t * 512 + ib * 128 : t * 512 + (ib + 1) * 128],
                    wo16[:, t * 512 : (t + 1) * 512],
                    start=(t == 0),
                    stop=False,
                )
            nc.tensor.matmul(ps[:], ones1[:], bo16[:], start=False, stop=True)
            nc.vector.tensor_copy(fin[:, ib * 512 : (ib + 1) * 512], ps[:])
        nc.sync.dma_start(out[s].rearrange("(a p) f -> p a f", p=128), fin[:])


def build_nc():
    nc = bacc.Bacc("TRN2", target_bir_lowering=False, debug=False)
    qT = nc.dram_tensor("qT", [S, DIM, N], BF16, kind="ExternalInput").ap()
    kvT = nc.dram_tensor("kvT", [S, DIM, N], BF16, kind="ExternalInput").ap()
    wq = nc.dram_tensor("wq", [DIM, DIM], BF16, kind="ExternalInput").ap()
    wk = nc.dram_tensor("wk", [DIM, DIM], BF16, kind="ExternalInput").ap()
    wv = nc.dram_tensor("wv", [DIM, DIM], BF16, kind="ExternalInput").ap()
    wo = nc.dram_tensor("wo", [DIM, DIM], BF16, kind="ExternalInput").ap()
    bo = nc.dram_tensor("bo", [DIM], BF16, kind="ExternalInput").ap()
    out = nc.dram_tensor("out", [S, N, DIM], F32, kind="ExternalOutput").ap()
    with tile.TileContext(nc) as tc:
        with ExitStack() as ctx:
            _build_body(ctx, tc, qT, kvT, wq, wk, wv, wo, bo, out)
    nc.compile()
    return nc


_NC = None
BF = ml_dtypes.bfloat16


def make_in_maps(q_in, kv_in, Wq, Wk, Wv, Wo, bo):
    # host-side layout prep: per-slice transpose to [dim, n] + bf16 cast
    q = np.asarray(q_in, dtype=np.float32).reshape(32, N, DIM)
    kv = np.asarray(kv_in, dtype=np.float32).reshape(32, N, DIM)
    qT = np.ascontiguousarray(q.transpose(0, 2, 1)).astype(BF)
    kvT = np.ascontiguousarray(kv.transpose(0, 2, 1)).astype(BF)
    w = {
        "wq": np.asarray(Wq, dtype=np.float32).astype(BF),
        "wk": np.asarray(Wk, dtype=np.float32).astype(BF),
        "wv": np.asarray(Wv, dtype=np.float32).astype(BF),
        "wo": np.asarray(Wo, dtype=np.float32).astype(BF),
        "bo": np.asarray(bo, dtype=np.float32).astype(BF),
    }
    return [
        {"qT": qT[S * c : S * (c + 1)], "kvT": kvT[S * c : S * (c + 1)], **w}
        for c in range(N_CORES)
    ]


def kernel(q_in, kv_in, Wq, Wk, Wv, Wo, bo):
    global _NC
    if _NC is None:
        _NC = build_nc()
    in_maps = make_in_maps(q_in, kv_in, Wq, Wk, Wv, Wo, bo)
    res = run_bass_kernel_spmd(_NC, in_maps, list(range(N_CORES))).results
    out = np.concatenate([res[c]["out"] for c in range(N_CORES)], axis=0)
    return out.reshape(4, 8, N, DIM)



# revision 4
# speedup vs baseline: 1.0432x; 1.0432x over previous
"""Trainium2 Bass kernel for CrossAttention (b=4, p=8, n=512, dim=512, 8 heads x 64).

Sharding: 32 independent (b, p) slices, 4 per core across 8 NeuronCores (pure
data parallel, no collectives). Weights replicated. Host transposes inputs to
[dim, n] bf16 per slice; host adds the output bias (exact, fp32).

Per-slice device dataflow (PE program order is fully software-pipelined):
  - qT/kT = Wq/Wk-blocks^T @ xT (bf16 matmuls, f32 PSUM) -> SBUF f32
    (read back as f32r by the score matmuls: full-precision scores at bf16 cost)
  - v = xkvT-blocks^T @ Wv -> SBUF bf16 in an augmented layout [., jb, head, 65]
    whose 65th column is constant 1.0: the PV matmul then produces the softmax
    denominator l in PSUM row 64 for free (no separate ones-matmul).
  - scores ST[j,i] per head (f32r) -> batched exp (ACT, scale=1/8, bias=-4,
    cancels in normalization) -> P^T bf16
  - PV: outT_h[d,i] rows 0..63 + l row 64, single accumulation group
  - normalize: Pool partition_broadcast of l + DVE divide -> outT bf16
  - fin[i,f] = outT-blocks^T @ Wo -> f32 -> DRAM (bias added on host)

Schedule: iteration i runs attention(slice i) on PE, with projections(i+1) and
fin(i-1) matmuls interleaved as filler between scores(h) and PV(h-1) so the PE
never waits on the exp/normalize pipelines (PV lags scores by one head).
"""

from contextlib import ExitStack

import ml_dtypes
import numpy as np

import concourse.bass as bass
import concourse.tile as tile
from concourse import bacc, mybir
from concourse.bass_utils import run_bass_kernel_spmd

F32 = mybir.dt.float32
F32R = mybir.dt.float32r
BF16 = mybir.dt.bfloat16
EXP = mybir.ActivationFunctionType.Exp

HEADS = 8
DH = 64
N = 512
DIM = 512
SCALE = DH**-0.5
EXP_SHIFT = -4.0  # exp(S/8 - 4): cancels in P/l; keeps exp outputs small
S = 4  # (b, p) slices per core
N_CORES = 8


def _build_body(ctx: ExitStack, tc: tile.TileContext, qT, kvT, wq, wk, wv, wo, out):
    nc = tc.nc

    const = ctx.enter_context(tc.tile_pool(name="const", bufs=1))
    xT = ctx.enter_context(tc.tile_pool(name="xT", bufs=1))  # unique names per slice
    proj32 = ctx.enter_context(tc.tile_pool(name="proj32", bufs=2))
    vap = ctx.enter_context(tc.tile_pool(name="vap", bufs=2))
    ptp = ctx.enter_context(tc.tile_pool(name="ptp", bufs=3))
    outTp = ctx.enter_context(tc.tile_pool(name="outTp", bufs=2))
    lbp = ctx.enter_context(tc.tile_pool(name="lbp", bufs=2))
    finp = ctx.enter_context(tc.tile_pool(name="finp", bufs=2))
    mm_ps = ctx.enter_context(tc.tile_pool(name="mm_ps", bufs=2, space="PSUM"))
    st_ps = ctx.enter_context(tc.tile_pool(name="st_ps", bufs=2, space="PSUM"))
    pv_ps = ctx.enter_context(tc.tile_pool(name="pv_ps", bufs=2, space="PSUM"))

    # --- DMA issue order tuned for startup: wq, xq0, wk, xkv0, wv, wo, rest.
    # The DMA device serializes transfers, so the first q-projection tile can
    # start after just wq+xq0 instead of after every weight.
    ebias = const.tile([128, 1], F32, name="ebias")
    nc.gpsimd.memset(ebias[:], EXP_SHIFT)
    w_sb = {
        name: const.tile([128, 4 * 512], BF16, name=f"{name}16")
        for name in ("wq", "wk", "wv", "wo")
    }
    xq_t = [xT.tile([128, 4 * 512], BF16, name=f"xqT{s}") for s in range(S)]
    xkv_t = [xT.tile([128, 4 * 512], BF16, name=f"xkvT{s}") for s in range(S)]

    def load_w(name, dram):
        nc.sync.dma_start(w_sb[name][:], dram.rearrange("(t p) e -> p t e", p=128))

    def load_x(dst, dram, s):
        nc.sync.dma_start(dst[s][:], dram[s].rearrange("(t p) n -> p t n", p=128))

    load_w("wq", wq)
    load_x(xq_t, qT, 0)
    load_w("wk", wk)
    load_x(xkv_t, kvT, 0)
    load_w("wv", wv)
    load_w("wo", wo)
    for s in range(1, S):
        load_x(xq_t, qT, s)
        load_x(xkv_t, kvT, s)
    wq16, wk16, wv16, wo16 = (w_sb[k] for k in ("wq", "wk", "wv", "wo"))

    # per-slice live tiles, filled by the filler closures
    qT32 = [None] * S
    kT32 = [None] * S
    v16a = [None] * S
    outT16 = [None] * S
    fin32 = [None] * S

    def proj_tiles(s):
        """12 closures: q/k t-blocks interleaved (earliest-need order), then v."""
        qs = proj32.tile([128, 4 * 512], F32R, name="qT32")
        ks = proj32.tile([128, 4 * 512], F32R, name="kT32")
        va = vap.tile([128, 4 * 520], BF16, name="v16a")
        qT32[s], kT32[s], v16a[s] = qs, ks, va
        va4 = va[:].rearrange("p (j h c) -> p j h c", j=4, h=8)
        nc.gpsimd.memset(va4[:, :, :, 64:65], 1.0)

        def qk_tile(w16, xt, dst, t):
            def go():
                ps = mm_ps.tile([128, 512], F32, name="mm_ps")
                for d in range(4):
                    nc.tensor.matmul(
                        ps[:],
                        w16[:, d * 512 + t * 128 : d * 512 + (t + 1) * 128],
                        xt[:, d * 512 : (d + 1) * 512],
                        start=(d == 0),
                        stop=(d == 3),
                    )
                nc.vector.tensor_copy(dst[:, t * 512 : (t + 1) * 512], ps[:])
            return go

        def v_tile(jb):
            def go():
                ps = mm_ps.tile([128, 512], F32, name="mm_ps")
                for d in range(4):
                    nc.tensor.matmul(
                        ps[:],
                        xkv_t[s][:, d * 512 + jb * 128 : d * 512 + (jb + 1) * 128],
                        wv16[:, d * 512 : (d + 1) * 512],
                        start=(d == 0),
                        stop=(d == 3),
                    )
                nc.vector.tensor_copy(va4[:, jb, :, 0:64], ps[:])
            return go

        tiles = []
        if s == 0:
            # prologue order: all-q first — only wq+xq0 need to have landed
            for t in range(4):
                tiles.append(qk_tile(wq16, xq_t[s], qs, t))
            for t in range(4):
                tiles.append(qk_tile(wk16, xkv_t[s], ks, t))
        else:
            for t in range(4):
                tiles.append(qk_tile(wq16, xq_t[s], qs, t))
                tiles.append(qk_tile(wk16, xkv_t[s], ks, t))
        for jb in range(4):
            tiles.append(v_tile(jb))
        return tiles

    def fin_tiles(s):
        """4 closures: fin ib-blocks. Mid-kernel slices stage through SBUF and
        ship one big DMA; the last slice DMAs each PSUM block directly to DRAM
        to shorten the drain tail."""
        direct = s == S - 1
        fs = finp.tile([128, 4 * 512], F32, name="fin32")
        fin32[s] = fs

        def fin_tile(ib):
            def go():
                ps = mm_ps.tile([128, 512], F32, name="mm_ps")
                oT = outT16[s]
                for t in range(4):
                    nc.tensor.matmul(
                        ps[:],
                        oT[:, t * 512 + ib * 128 : t * 512 + (ib + 1) * 128],
                        wo16[:, t * 512 : (t + 1) * 512],
                        start=(t == 0),
                        stop=(t == 3),
                    )
                nc.vector.tensor_copy(fs[:, ib * 512 : (ib + 1) * 512], ps[:])
                if direct:
                    nc.sync.dma_start(
                        out[s][ib * 128 : (ib + 1) * 128, :],
                        fs[:, ib * 512 : (ib + 1) * 512],
                    )
                elif ib == 3:
                    nc.sync.dma_start(
                        out[s].rearrange("(a p) f -> p a f", p=128), fs[:]
                    )
            return go

        return [fin_tile(ib) for ib in range(4)]

    pt16 = [[None] * HEADS for _ in range(S)]

    def scores_exp(s, h):
        half, tp = (h % 2) * 64, h // 2
        q_h = qT32[s][half : half + 64, tp * 512 : (tp + 1) * 512]
        pt = ptp.tile([128, 4 * 512], BF16, name="pt16")
        pt16[s][h] = pt
        for half_jb in range(2):  # two [128,1024] score tiles -> batched exp
            st = st_ps.tile([128, 1024], F32, name="st_ps")
            for j in range(2):
                jb = half_jb * 2 + j
                k_h = kT32[s][
                    half : half + 64, tp * 512 + jb * 128 : tp * 512 + (jb + 1) * 128
                ]
                nc.tensor.matmul(
                    st[:, j * 512 : (j + 1) * 512], k_h, q_h, start=True, stop=True
                )
            nc.scalar.activation(
                pt[:, half_jb * 1024 : (half_jb + 1) * 1024],
                st[:],
                EXP,
                scale=SCALE,
                bias=ebias[:],
            )

    def pv_norm(s, h):
        half, tp = (h % 2) * 64, h // 2
        va4 = v16a[s][:].rearrange("p (j h c) -> p j h c", j=4, h=8)
        pt = pt16[s][h]
        pv = pv_ps.tile([128, 512], F32, name="pv_ps")
        for jb in range(4):
            nc.tensor.matmul(
                pv[0:65, :],
                va4[:, jb, h, :],
                pt[:, jb * 512 : (jb + 1) * 512],
                start=(jb == 0),
                stop=(jb == 3),
            )
        # GPSIMD cannot read PSUM: reciprocal on DVE (PSUM->SBUF), broadcast on
        # Pool (SBUF->SBUF), multiply on DVE.
        rl = lbp.tile([1, 512], F32, name="rl")
        nc.vector.reciprocal(rl[:], pv[64:65, :])
        lb = lbp.tile([64, 512], F32, name="lb")
        nc.gpsimd.partition_broadcast(lb[:], rl[:])
        nc.vector.tensor_tensor(
            outT16[s][half : half + 64, tp * 512 : (tp + 1) * 512],
            pv[0:64, :],
            lb[:],
            mybir.AluOpType.mult,
        )

    # ---- prologue: projections for slice 0 ----
    for go in proj_tiles(0):
        go()

    # ---- pipelined main loop ----
    for i in range(S):
        outT16[i] = outTp.tile([128, 4 * 512], BF16, name="outT16")
        filler = list(proj_tiles(i + 1)) if i + 1 < S else [None] * 12
        filler += fin_tiles(i - 1) if i > 0 else [None] * 4
        fq = iter(filler)
        for h in range(HEADS):
            scores_exp(i, h)
            for _ in range(2):
                go = next(fq, None)
                if go is not None:
                    go()  # None = no filler for this slot (first/last iteration)
            if h > 0:
                pv_norm(i, h - 1)
            elif i > 0:
                pv_norm(i - 1, HEADS - 1)
        for go in fq:
            go()

    # ---- epilogue ----
    pv_norm(S - 1, HEADS - 1)
    for go in fin_tiles(S - 1):
        go()


def build_nc():
    nc = bacc.Bacc("TRN2", target_bir_lowering=False, debug=False)
    qT = nc.dram_tensor("qT", [S, DIM, N], BF16, kind="ExternalInput").ap()
    kvT = nc.dram_tensor("kvT", [S, DIM, N], BF16, kind="ExternalInput").ap()
    wq = nc.dram_tensor("wq", [DIM, DIM], BF16, kind="ExternalInput").ap()
    wk = nc.dram_tensor("wk", [DIM, DIM], BF16, kind="ExternalInput").ap()
    wv = nc.dram_tensor("wv", [DIM, DIM], BF16, kind="ExternalInput").ap()
    wo = nc.dram_tensor("wo", [DIM, DIM], BF16, kind="ExternalInput").ap()
    out = nc.dram_tensor("out", [S, N, DIM], F32, kind="ExternalOutput").ap()
    with tile.TileContext(nc) as tc:
        with ExitStack() as ctx:
            _build_body(ctx, tc, qT, kvT, wq, wk, wv, wo, out)
    nc.compile()
    return nc


_NC = None
BF = ml_dtypes.bfloat16


def make_in_maps(q_in, kv_in, Wq, Wk, Wv, Wo, bo):
    q = np.asarray(q_in, dtype=np.float32).reshape(32, N, DIM)
    kv = np.asarray(kv_in, dtype=np.float32).reshape(32, N, DIM)
    qT = np.ascontiguousarray(q.transpose(0, 2, 1)).astype(BF)
    kvT = np.ascontiguousarray(kv.transpose(0, 2, 1)).astype(BF)
    w = {
        "wq": np.asarray(Wq, dtype=np.float32).astype(BF),
        "wk": np.asarray(Wk, dtype=np.float32).astype(BF),
        "wv": np.asarray(Wv, dtype=np.float32).astype(BF),
        "wo": np.asarray(Wo, dtype=np.float32).astype(BF),
    }
    return [
        {"qT": qT[S * c : S * (c + 1)], "kvT": kvT[S * c : S * (c + 1)], **w}
        for c in range(N_CORES)
    ]


def kernel(q_in, kv_in, Wq, Wk, Wv, Wo, bo):
    global _NC
    if _NC is None:
        _NC = build_nc()
    in_maps = make_in_maps(q_in, kv_in, Wq, Wk, Wv, Wo, bo)
    res = run_bass_kernel_spmd(_NC, in_maps, list(range(N_CORES))).results
    out = np.concatenate([res[c]["out"] for c in range(N_CORES)], axis=0)
    out = out + np.asarray(bo, dtype=np.float32)[None, None, :]
    return out.reshape(4, 8, N, DIM)


# revision 5
# speedup vs baseline: 1.1277x; 1.0810x over previous
"""Trainium2 Bass kernel for CrossAttention (b=4, p=8, n=512, dim=512, 8 heads x 64).

Sharding: 32 independent (b, p) slices, 4 per core across 8 NeuronCores (pure
data parallel, no collectives). Weights replicated. Host transposes inputs to
[dim, n] bf16 per slice; host adds the output bias (exact, fp32).

Per-slice device dataflow (PE program order is fully software-pipelined):
  - qT/kT = Wq/Wk-blocks^T @ xT (bf16 matmuls, f32 PSUM) -> SBUF f32
    (read back as f32r by the score matmuls: full-precision scores at bf16 cost)
  - v = xkvT-blocks^T @ Wv -> SBUF bf16 in an augmented layout [., jb, head, 65]
    whose 65th column is constant 1.0: the PV matmul then produces the softmax
    denominator l in PSUM row 64 for free (no separate ones-matmul).
  - scores ST[j,i] per head (f32r) -> batched exp (ACT, scale=1/8, bias=-4,
    cancels in normalization) -> P^T bf16
  - PV: outT_h[d,i] rows 0..63 + l row 64, single accumulation group
  - normalize: Pool partition_broadcast of l + DVE divide -> outT bf16
  - fin[i,f] = outT-blocks^T @ Wo -> f32 -> DRAM (bias added on host)

Schedule: iteration i runs attention(slice i) on PE, with projections(i+1) and
fin(i-1) matmuls interleaved as filler between scores(h) and PV(h-1) so the PE
never waits on the exp/normalize pipelines (PV lags scores by one head).
"""

from contextlib import ExitStack

import ml_dtypes
import numpy as np

import concourse.bass as bass
import concourse.tile as tile
from concourse import bacc, mybir
from concourse.bass_utils import run_bass_kernel_spmd

F32 = mybir.dt.float32
F32R = mybir.dt.float32r
BF16 = mybir.dt.bfloat16
EXP = mybir.ActivationFunctionType.Exp

HEADS = 8
DH = 64
N = 512
DIM = 512
SCALE = DH**-0.5
EXP_SHIFT = -4.0  # exp(S/8 - 4): cancels in P/l; keeps exp outputs small
S = 4  # (b, p) slices per core
WARMUPS = 0
N_CORES = 8


def _build_body(ctx: ExitStack, tc: tile.TileContext, qT, kvT, wq, wk, wv, wo, out):
    nc = tc.nc

    const = ctx.enter_context(tc.tile_pool(name="const", bufs=1))
    xT = ctx.enter_context(tc.tile_pool(name="xT", bufs=1))  # unique names per slice
    proj32 = ctx.enter_context(tc.tile_pool(name="proj32", bufs=2))
    vap = ctx.enter_context(tc.tile_pool(name="vap", bufs=2))
    ptp = ctx.enter_context(tc.tile_pool(name="ptp", bufs=3))
    outTp = ctx.enter_context(tc.tile_pool(name="outTp", bufs=2))
    lbp = ctx.enter_context(tc.tile_pool(name="lbp", bufs=2))
    finp = ctx.enter_context(tc.tile_pool(name="finp", bufs=2))
    mm_ps = ctx.enter_context(tc.tile_pool(name="mm_ps", bufs=2, space="PSUM"))
    st_ps = ctx.enter_context(tc.tile_pool(name="st_ps", bufs=2, space="PSUM"))
    pv_ps = ctx.enter_context(tc.tile_pool(name="pv_ps", bufs=2, space="PSUM"))

    # --- DMA issue order tuned for startup: wq, xq0, wk, xkv0, wv, wo, rest.
    # The DMA device serializes transfers, so the first q-projection tile can
    # start after just wq+xq0 instead of after every weight.
    ebias = const.tile([128, 1], F32, name="ebias")
    nc.gpsimd.memset(ebias[:], EXP_SHIFT)
    w_sb = {
        name: const.tile([128, 4 * 512], BF16, name=f"{name}16")
        for name in ("wq", "wk", "wv", "wo")
    }
    xq_t = [xT.tile([128, 4 * 512], BF16, name=f"xqT{s}") for s in range(S)]
    xkv_t = [xT.tile([128, 4 * 512], BF16, name=f"xkvT{s}") for s in range(S)]

    def load_w(name, dram):
        nc.sync.dma_start(w_sb[name][:], dram.rearrange("(t p) e -> p t e", p=128))

    def load_x(dst, dram, s):
        nc.sync.dma_start(dst[s][:], dram[s].rearrange("(t p) n -> p t n", p=128))

    # First wq/xq0 loads are split per K-chunk so the very first projection
    # matmuls can chase the DMA chunks instead of waiting for whole tiles.
    wqr = wq.rearrange("(t p) e -> t p e", p=128)
    xq0r = qT[0].rearrange("(t p) n -> t p n", p=128)
    for c in range(4):
        nc.sync.dma_start(w_sb["wq"][:, c * 512 : (c + 1) * 512], wqr[c])
        nc.sync.dma_start(xq_t[0][:, c * 512 : (c + 1) * 512], xq0r[c])
    load_w("wk", wk)
    load_x(xkv_t, kvT, 0)
    load_w("wv", wv)
    load_w("wo", wo)
    for s in range(1, S):
        load_x(xq_t, qT, s)
        load_x(xkv_t, kvT, s)
    wq16, wk16, wv16, wo16 = (w_sb[k] for k in ("wq", "wk", "wv", "wo"))

    # PE p-state warmup: harmless matmuls on a zeroed tile keep the PE busy
    # from t~0 so the ramp (slow first ~3us) is spent before real work lands.
    warm = const.tile([128, 512], BF16, name="warm")
    nc.gpsimd.memset(warm[:], 0.0)
    warm_ps = mm_ps.tile([128, 512], F32, name="mm_ps")
    for _ in range(WARMUPS):
        nc.tensor.matmul(warm_ps[:], warm[:, 0:128], warm[:], start=True, stop=True)

    # per-slice live tiles, filled by the filler closures
    qT32 = [None] * S
    kT32 = [None] * S
    v16a = [None] * S
    outT16 = [None] * S
    fin32 = [None] * S

    def proj_tiles(s):
        """12 closures: q/k t-blocks interleaved (earliest-need order), then v."""
        qs = proj32.tile([128, 4 * 512], F32R, name="qT32")
        ks = proj32.tile([128, 4 * 512], F32R, name="kT32")
        va = vap.tile([128, 4 * 520], BF16, name="v16a")
        qT32[s], kT32[s], v16a[s] = qs, ks, va
        va4 = va[:].rearrange("p (j h c) -> p j h c", j=4, h=8)
        nc.gpsimd.memset(va4[:, :, :, 64:65], 1.0)

        def qk_tile(w16, xt, dst, t):
            def go():
                ps = mm_ps.tile([128, 512], F32, name="mm_ps")
                for d in range(4):
                    nc.tensor.matmul(
                        ps[:],
                        w16[:, d * 512 + t * 128 : d * 512 + (t + 1) * 128],
                        xt[:, d * 512 : (d + 1) * 512],
                        start=(d == 0),
                        stop=(d == 3),
                    )
                nc.vector.tensor_copy(dst[:, t * 512 : (t + 1) * 512], ps[:])
            return go

        def v_tile(jb):
            def go():
                ps = mm_ps.tile([128, 512], F32, name="mm_ps")
                for d in range(4):
                    nc.tensor.matmul(
                        ps[:],
                        xkv_t[s][:, d * 512 + jb * 128 : d * 512 + (jb + 1) * 128],
                        wv16[:, d * 512 : (d + 1) * 512],
                        start=(d == 0),
                        stop=(d == 3),
                    )
                nc.vector.tensor_copy(va4[:, jb, :, 0:64], ps[:])
            return go

        tiles = []
        if s == 0:
            # prologue order: all-q first — only wq+xq0 need to have landed
            for t in range(4):
                tiles.append(qk_tile(wq16, xq_t[s], qs, t))
            for t in range(4):
                tiles.append(qk_tile(wk16, xkv_t[s], ks, t))
        else:
            for t in range(4):
                tiles.append(qk_tile(wq16, xq_t[s], qs, t))
                tiles.append(qk_tile(wk16, xkv_t[s], ks, t))
        for jb in range(4):
            tiles.append(v_tile(jb))
        return tiles

    def fin_tiles(s):
        """4 closures: fin ib-blocks. Mid-kernel slices stage through SBUF and
        ship one big DMA; the last slice DMAs each PSUM block directly to DRAM
        to shorten the drain tail."""
        direct = s == S - 1
        fs = finp.tile([128, 4 * 512], F32, name="fin32")
        fin32[s] = fs

        def fin_tile(ib):
            def go():
                ps = mm_ps.tile([128, 512], F32, name="mm_ps")
                oT = outT16[s]
                for t in range(4):
                    nc.tensor.matmul(
                        ps[:],
                        oT[:, t * 512 + ib * 128 : t * 512 + (ib + 1) * 128],
                        wo16[:, t * 512 : (t + 1) * 512],
                        start=(t == 0),
                        stop=(t == 3),
                    )
                nc.vector.tensor_copy(fs[:, ib * 512 : (ib + 1) * 512], ps[:])
                if direct:
                    nc.sync.dma_start(
                        out[s][ib * 128 : (ib + 1) * 128, :],
                        fs[:, ib * 512 : (ib + 1) * 512],
                    )
                elif ib == 3:
                    nc.sync.dma_start(
                        out[s].rearrange("(a p) f -> p a f", p=128), fs[:]
                    )
            return go

        return [fin_tile(ib) for ib in range(4)]

    pt16 = [[None] * HEADS for _ in range(S)]

    def scores_exp(s, h):
        half, tp = (h % 2) * 64, h // 2
        q_h = qT32[s][half : half + 64, tp * 512 : (tp + 1) * 512]
        pt = ptp.tile([128, 4 * 512], BF16, name="pt16")
        pt16[s][h] = pt
        for half_jb in range(2):  # two [128,1024] score tiles -> batched exp
            st = st_ps.tile([128, 1024], F32, name="st_ps")
            for j in range(2):
                jb = half_jb * 2 + j
                k_h = kT32[s][
                    half : half + 64, tp * 512 + jb * 128 : tp * 512 + (jb + 1) * 128
                ]
                nc.tensor.matmul(
                    st[:, j * 512 : (j + 1) * 512], k_h, q_h, start=True, stop=True
                )
            nc.scalar.activation(
                pt[:, half_jb * 1024 : (half_jb + 1) * 1024],
                st[:],
                EXP,
                scale=SCALE,
                bias=ebias[:],
            )

    def pv_norm(s, h):
        half, tp = (h % 2) * 64, h // 2
        va4 = v16a[s][:].rearrange("p (j h c) -> p j h c", j=4, h=8)
        pt = pt16[s][h]
        pv = pv_ps.tile([128, 512], F32, name="pv_ps")
        for jb in range(4):
            nc.tensor.matmul(
                pv[0:65, :],
                va4[:, jb, h, :],
                pt[:, jb * 512 : (jb + 1) * 512],
                start=(jb == 0),
                stop=(jb == 3),
            )
        # GPSIMD cannot read PSUM: reciprocal on DVE (PSUM->SBUF), broadcast on
        # Pool (SBUF->SBUF), multiply on DVE.
        rl = lbp.tile([1, 512], F32, name="rl")
        nc.vector.reciprocal(rl[:], pv[64:65, :])
        lb = lbp.tile([64, 512], F32, name="lb")
        nc.gpsimd.partition_broadcast(lb[:], rl[:])
        nc.vector.tensor_tensor(
            outT16[s][half : half + 64, tp * 512 : (tp + 1) * 512],
            pv[0:64, :],
            lb[:],
            mybir.AluOpType.mult,
        )

    # ---- prologue: projections for slice 0 ----
    for go in proj_tiles(0):
        go()

    # ---- pipelined main loop ----
    for i in range(S):
        outT16[i] = outTp.tile([128, 4 * 512], BF16, name="outT16")
        filler = list(proj_tiles(i + 1)) if i + 1 < S else [None] * 12
        filler += fin_tiles(i - 1) if i > 0 else [None] * 4
        fq = iter(filler)
        for h in range(HEADS):
            scores_exp(i, h)
            for _ in range(2):
                go = next(fq, None)
                if go is not None:
                    go()  # None = no filler for this slot (first/last iteration)
            if h > 0:
                pv_norm(i, h - 1)
            elif i > 0:
                pv_norm(i - 1, HEADS - 1)
        for go in fq:
            go()

    # ---- epilogue ----
    pv_norm(S - 1, HEADS - 1)
    for go in fin_tiles(S - 1):
        go()


def build_nc():
    nc = bacc.Bacc("TRN2", target_bir_lowering=False, debug=False)
    qT = nc.dram_tensor("qT", [S, DIM, N], BF16, kind="ExternalInput").ap()
    kvT = nc.dram_tensor("kvT", [S, DIM, N], BF16, kind="ExternalInput").ap()
    wq = nc.dram_tensor("wq", [DIM, DIM], BF16, kind="ExternalInput").ap()
    wk = nc.dram_tensor("wk", [DIM, DIM], BF16, kind="ExternalInput").ap()
    wv = nc.dram_tensor("wv", [DIM, DIM], BF16, kind="ExternalInput").ap()
    wo = nc.dram_tensor("wo", [DIM, DIM], BF16, kind="ExternalInput").ap()
    out = nc.dram_tensor("out", [S, N, DIM], F32, kind="ExternalOutput").ap()
    with tile.TileContext(nc) as tc:
        with ExitStack() as ctx:
            _build_body(ctx, tc, qT, kvT, wq, wk, wv, wo, out)
    nc.compile()
    return nc


_NC = None
BF = ml_dtypes.bfloat16


def make_in_maps(q_in, kv_in, Wq, Wk, Wv, Wo, bo):
    q = np.asarray(q_in, dtype=np.float32).reshape(32, N, DIM)
    kv = np.asarray(kv_in, dtype=np.float32).reshape(32, N, DIM)
    qT = np.ascontiguousarray(q.transpose(0, 2, 1)).astype(BF)
    kvT = np.ascontiguousarray(kv.transpose(0, 2, 1)).astype(BF)
    w = {
        "wq": np.asarray(Wq, dtype=np.float32).astype(BF),
        "wk": np.asarray(Wk, dtype=np.float32).astype(BF),
        "wv": np.asarray(Wv, dtype=np.float32).astype(BF),
        "wo": np.asarray(Wo, dtype=np.float32).astype(BF),
    }
    return [
        {"qT": qT[S * c : S * (c + 1)], "kvT": kvT[S * c : S * (c + 1)], **w}
        for c in range(N_CORES)
    ]


def kernel(q_in, kv_in, Wq, Wk, Wv, Wo, bo):
    global _NC
    if _NC is None:
        _NC = build_nc()
    in_maps = make_in_maps(q_in, kv_in, Wq, Wk, Wv, Wo, bo)
    res = run_bass_kernel_spmd(_NC, in_maps, list(range(N_CORES))).results
    out = np.concatenate([res[c]["out"] for c in range(N_CORES)], axis=0)
    out = out + np.asarray(bo, dtype=np.float32)[None, None, :]
    return out.reshape(4, 8, N, DIM)
